# revision 7
# baseline (speedup 1.0000x reference)
"""Trainium2 Bass kernel for a 2-layer message-passing GNN (BaselineGNN).

Reference computation (N=4096 nodes, IN=512, HID=4096, E=65536 edges):
    h   = x @ We.T + be                                   [N, HID]
    for W, b in ((W1, b1), (W2, b2)):
        aggr = segment_sum(h[col], row)                   [N, HID]
        h    = relu(aggr @ W.T + b)
    hm  = mean(h, axis=1)                                 [N]
    z   = relu(hm @ Wc1.T + bc1)                          [HID//2]
    out = (z @ Wc2.T + bc2).squeeze(-1)                   scalar

Strategy (8 NeuronCores, node-parallel, deep collective pipelining):
  * segment_sum == A @ h with A the [N, N] adjacency-count matrix (0.4%
    dense).  A's entries are small integer counts -> exactly representable
    in fp8-e4m3, so aggregation runs as a dense TensorEngine matmul.
  * Nodes are sharded: core c owns rows 512c..512c+512.
  * Layer 1 is low-rank through the embed bottleneck:
        h1_c = relu((A_c @ x_ext) @ (We_ext.T @ W1.T) + b1)
    with the weight product folded on the host (x_ext carries a ones
    column, We_ext.T a b_embed row).
  * Layer 2 is REORDERED to hide the collective:
        h2 = relu(A @ (h1 @ W2.T) + b2)
    Each core computes p_c = h1_c @ W2.T locally (W2 replicated) BEFORE
    any cross-core exchange.  p is exchanged in NAG=4 column quarters:
    each quarter's AllGather fires as soon as M5 finishes that quarter,
    so all four AGs ride under the remaining M5 quarters and early M4
    quarters -- the PE never waits for the fabric.
  * Pipeline per core: M1 tT=(A_c@x_ext).T [fp8 DR, ONE matmul k-batched
    over 4 quarter-tiles so the first matmul depends on only ~1.1MB of
    DMA] -> M3 h1T [bf16, kxm streamed from two wcb half-tiles so M3
    starts when half the weight product has landed] -> M5 quarters
    p_c = h1T.T @ (W2.T*16) [fp8 DR] -> AllGather quarter (fp8) ->
    M4 quarters aggr2 = at8.T @ p_full [fp8 DR] with relu+row-sum fused
    into the PSUM eviction (ScalarE accum_out) -> hm directly.
  * M4's kxm side reuses M1's at8 quarter tiles (still resident in
    SBUF), so the adjacency matrix is DMAed exactly once.
  * The z head runs in 2 chunks (after M4 quarters 1 and 3); each chunk
    zp = Wc1_c @ hm_chunk is exchanged with an AllGather (half the
    latency of AllReduce = RS+AG) and summed locally on the DVE, so
    only the last small AG sits on the tail.
  * fp8 operands that are not exact (x, h1, p, W2) rely on fp32 PSUM
    accumulation and the mean-pool/classifier averaging to wash the 6%
    element-level rounding down to ~1e-3 relative error on the scalar
    output; W2 is pre-scaled by 16 to sit in e4m3's normal range (the
    hm normalization divides it back out).  The classifier runs bf16/fp32.
  * A dummy AllReduce issued first absorbs the multi-core launch skew on
    the collective engine while the PE computes M1/M3/M5.

Everything is expressed "transposed-free": every matmul is
mxn = kxm.T @ kxn with operands stored so no transposes are ever needed.
"""

import contextlib

import numpy as np
import ml_dtypes

import concourse.bass as bass
import concourse.mybir as mybir
import concourse.tile as tile
from concourse import bacc
from concourse.bass_interp import get_hw_module
from concourse.bass_utils import run_bass_kernel_spmd
from concourse.kernels.tile_matmul import (
    ShapeInfo,
    composable_matmul_tile_kernel,
    dma_from_dram_kxn,
    dma_to_dram_mxn,
    k_pool_min_bufs,
    scalar_copyback,
)

N = 4096          # nodes
IN_DIM = 512
HID = 4096
NCORES = 8
S = N // NCORES           # nodes per core (512)
KE = 640                  # extended embed contraction (512 + 1 ones col, padded to 5*128)
CHID = HID // 2           # classifier hidden (2048)

BF16 = mybir.dt.bfloat16
F32 = mybir.dt.float32
FP8 = mybir.dt.float8e4

USE_FP8 = True
# p = h1 @ (W2.T * WSCALE) is materialized in fp8-e4m3 (range +-448):
# p_true has absmax ~12, so WSCALE=16 centers it (~200 max) with margin.
WSCALE = 16.0
M3_FP8 = False
WCB_SCALE = 16.0
KEP = 768 if M3_FP8 else KE   # KE padded to a DoubleRow-even k (6*128)

NAG = 4                   # p AllGather column-chunk count
HQ = HID // NAG           # 1024
NZ = 2                    # z-exchange chunk count
NKQ = 8                   # M1/M4 k-chunk tiles (N // 512)
KQ = N // NKQ             # 512 rows per k-chunk
NWC = 8                   # wcb column chunks

_COMPILED = {}


def _m4_reducer(nc_b, bias_sb, accum, n_off):
    """PSUM->SBUF evict for the final aggregation:
    sbuf = relu(psum + b2[n_slice]); row-sums stream into accum.

    bias_sb: [128, HID] SBUF tile (b2 broadcast across partitions), or None
    when b2 is all-zero.  accum: [128, MSUB, NT] fp32; the relu
    row-sum-accumulates into accum[:, m_subtile, n_off + n_tile].
    """

    def _reducer(nc, psum, sbuf, md):
        src_ap = psum[:, : md.n_slice_size]
        ni = n_off + md.n_tile_idx * md.n_subtiles + md.n_subtile_idx
        if bias_sb is not None:
            start = (
                n_off * md.n_tile
                + md.n_tile_idx * md.n_tile
                + md.n_subtile_idx * md.n_subtile
            )
            nc.vector.tensor_add(
                out=sbuf[:, :, : md.n_slice_size],
                in0=src_ap,
                in1=bias_sb[:, start : start + md.n_slice_size],
            )
            src_ap = sbuf
        nc.scalar.activation(
            out=sbuf,
            in_=src_ap,
            func=mybir.ActivationFunctionType.Relu,
            accum_out=accum[:, md.m_subtile_idx, ni : ni + 1],
        )

    return _reducer


def _m3_reducer(nc_b, b1c_sb):
    """PSUM->SBUF evict for h1T: sbuf = relu(psum + b1[m_slice]).

    h1T is feature-major, so the layer-1 bias is per-partition:
    b1c_sb is [128, HID//128] with b1c[p, m] = b1[m*128 + p], or None
    when b1 is all-zero.
    """

    def _reducer(nc, psum, sbuf, md):
        if b1c_sb is None:
            nc.scalar.activation(
                out=sbuf, in_=psum, func=mybir.ActivationFunctionType.Relu
            )
        else:
            mi = md.m_tile_idx * md.m_subtiles + md.m_subtile_idx
            nc.vector.tensor_add(
                out=sbuf[:, :, : md.n_slice_size],
                in0=psum[:, : md.n_slice_size],
                in1=b1c_sb[:, mi : mi + 1].to_broadcast(
                    [128, 1, md.n_slice_size]
                ),
            )
            nc.vector.tensor_scalar_max(sbuf, sbuf, 0.0)

    return _reducer


def _build_graph(b1_zero=True, b2_zero=True):
    nc = bacc.Bacc(
        "TRN2",
        target_bir_lowering=False,
        debug=False,
        enable_asserts=False,
        num_devices=NCORES,
    )

    ADT = FP8 if USE_FP8 else BF16

    # ---- kernel I/O (per core) ----
    WBT = ADT if M3_FP8 else BF16
    xe = nc.dram_tensor("xe", [N, KEP], ADT, kind="ExternalInput")        # x_ext (replicated, padded)
    at8 = nc.dram_tensor("at8", [N, S], ADT, kind="ExternalInput")        # A.T[:, rows_c] (sharded)
    wcb = nc.dram_tensor("wcb", [KEP, HID], WBT, kind="ExternalInput")    # We_ext.T @ W1.T (replicated)
    w2 = nc.dram_tensor("w2", [HID, HID], ADT, kind="ExternalInput")      # W2.T * WSCALE (replicated)
    b1 = nc.dram_tensor("b1", [128, HID // 128], F32, kind="ExternalInput")  # b1 partition-major
    b2 = nc.dram_tensor("b2", [128, HID], F32, kind="ExternalInput")      # b2*WSCALE bcast (replicated)
    wc1 = nc.dram_tensor("wc1", [S, CHID], BF16, kind="ExternalInput")    # Wc1.T row-chunk (sharded)
    bc1 = nc.dram_tensor("bc1", [128, CHID // 128], F32, kind="ExternalInput")  # bc1 [128,16]
    wc2 = nc.dram_tensor("wc2", [128, CHID // 128], F32, kind="ExternalInput")  # Wc2 [128,16]
    res = nc.dram_tensor("res", [1, 1], F32, kind="ExternalOutput")       # final scalar (pre-bc2)

    # ---- internal DRAM ----
    # p = h1 @ (W2.T*16) is exchanged in column quarters so each AllGather
    # overlaps with the production of the next quarter (M5) / consumption
    # of previous ones (M4).
    p_c = [nc.dram_tensor(f"pc{i}", [S, HQ], ADT) for i in range(NAG)]
    p_f = [
        nc.dram_tensor(f"pf{i}", [N, HQ], ADT, addr_space="Shared")
        for i in range(NAG)
    ]
    zgb = [nc.dram_tensor(f"zgb{i}", [1, CHID], F32) for i in range(NZ)]
    zgf = [
        nc.dram_tensor(f"zgf{i}", [NCORES, CHID], F32, addr_space="Shared")
        for i in range(NZ)
    ]
    da = nc.dram_tensor("da", [1, 8], F32)              # launch-skew sync dummy
    df = nc.dram_tensor("df", [1, 8], F32, addr_space="Shared")

    MSUB = S // 128   # 4 m-subtiles in a 512-row tile
    NT = HID // 512   # 8 hm column groups of 512
    NTQ = HQ // 512   # n-tiles (hm groups) per AG quarter (2)
    NTZ = NT // NZ    # hm groups per z chunk (4)
    KSQ = KQ // 128   # k-subtiles per k-quarter tile (8)

    with tile.TileContext(nc) as tc:
        with contextlib.ExitStack() as octx:
            const = octx.enter_context(tc.tile_pool(name="const", bufs=1))
            b2_sb = (
                None if b2_zero else const.tile([128, HID], F32, name="b2_sb")
            )
            b1c_sb = const.tile([128, HID // 128], F32, name="b1c_sb")
            hm_parts = const.tile([128, MSUB, NT], F32, name="hm_parts")
            nc.any.memset(hm_parts[:], 0.0)

            head = octx.enter_context(tc.tile_pool(name="head", bufs=1))
            CI = CHID // 128  # 16
            wc1_t = head.tile([128, MSUB, CHID], BF16, name="wc1_t")
            bc1_t = head.tile([128, CI], F32, name="bc1_t")
            wc2_t = head.tile([128, CI], F32, name="wc2_t")
            zp_t = [head.tile([1, CHID], F32, name=f"zp_t{i}") for i in range(NZ)]
            hm_i = [head.tile([128, MSUB], F32, name=f"hm_i{i}") for i in range(NZ)]
            hm_ib = [head.tile([128, MSUB], BF16, name=f"hm_ib{i}") for i in range(NZ)]
            zg_t = [
                head.tile([128, NCORES, CI], F32, name=f"zg_t{i}")
                for i in range(NZ)
            ]
            z2_t = head.tile([128, CI], F32, name="z2_t")
            zcol_t = head.tile([128, 1], F32, name="zcol_t")
            ones_t = head.tile([128, 1], F32, name="ones_t")
            r_t = head.tile([1, 1], F32, name="r_t")
            nc.any.memset(ones_t[:], 1.0)

            # persistent SBUF caches for chained-matmul intermediates.
            # at8 quarters stay resident: M1's kxn side AND M4's kxm side.
            cache = octx.enter_context(tc.tile_pool(name="cache", bufs=1))
            at8_q = [
                cache.tile([128, KSQ, S], ADT, name=f"at8_q{i}")
                for i in range(NKQ)
            ]
            tT_c = cache.tile([128, KEP // 128, S], WBT, name="tT_c")
            h1T_c = cache.tile([128, HID // 128, S], ADT, name="h1T_c")
            # wcb in column chunks (one per M3 m-tile) so M3's first
            # m-tiles depend on only ~0.6MB of the 5MB weight product
            WCW = HID // NWC  # 512
            wcb_c = [
                cache.tile([128, KEP // 128, WCW], WBT, name=f"wcb_c{i}")
                for i in range(NWC)
            ]
            # xe chunks live only until M1 completes
            m1ctx = contextlib.ExitStack()
            m1pool = m1ctx.enter_context(tc.tile_pool(name="m1pool", bufs=1))
            xe_q = [
                m1pool.tile([128, KSQ, KEP], ADT, name=f"xe_q{i}")
                for i in range(NKQ)
            ]
            at8_r = at8[:, :].rearrange("(po pi) n -> pi po n", pi=128)
            xe_r = xe[:, :].rearrange("(po pi) n -> pi po n", pi=128)
            wcb_r = wcb[:, :].rearrange("(po pi) n -> pi po n", pi=128)
            # interleave chunk loads, alternating rings per k-chunk so both
            # rings deliver M1's operands in consumption order; the first
            # matmul depends on only ~0.6MB
            for q in range(NKQ):
                po = q * KSQ
                ea = nc.sync if q % 2 == 0 else nc.scalar
                eb = nc.scalar if q % 2 == 0 else nc.sync
                ea.dma_start(
                    out=at8_q[q][:, :, :], in_=at8_r[:, po : po + KSQ, :]
                )
                eb.dma_start(
                    out=xe_q[q][:, :, :], in_=xe_r[:, po : po + KSQ, :]
                )
            # wcb chunks next, alternating rings, in M3 m-tile order
            for h in range(NWC):
                eng = nc.sync if h % 2 == 0 else nc.scalar
                eng.dma_start(
                    out=wcb_c[h][:, :, :],
                    in_=wcb_r[:, :, h * WCW : (h + 1) * WCW],
                )
            noop = lambda nc_, sbuf, md: None

            # dummy AllReduce fired first: absorbs the multi-core launch skew
            # on the collective engine while the PE is busy with M1/M3/M5, so
            # the p AllGathers later only see compute drift
            sync_t = head.tile([1, 8], F32, name="sync_t")
            nc.any.memset(sync_t[:], 0.0)
            nc.sync.dma_start(out=da[:, :], in_=sync_t[:, :])
            nc.gpsimd.collective_compute(
                "AllReduce",
                mybir.AluOpType.add,
                ins=[da[:, :].opt()],
                outs=[df[:, :].opt()],
                replica_groups=[list(range(NCORES))],
            )

            # M1: tT = (A_c @ x_ext).T = xe.T @ A_c.T        [KEP, S]
            # one matmul, k-batched over the 4 quarter tiles (PSUM
            # accumulates across k-tiles), so no combine pass is needed and
            # the first matmul depends on only the first quarters' DMAs
            k_shape = ShapeInfo(pdims=((128, N // 128),), fdims=(KEP,))
            n_shape = ShapeInfo(pdims=((128, N // 128),), fdims=(S,))

            def xe_producer(nc_, md):
                return xe_q[md.k_tile_idx][
                    :, :, md.m_tile_idx * md.m_tile : (md.m_tile_idx + 1) * md.m_tile
                ]

            def at8_producer_kxn(nc_, md):
                return at8_q[md.k_tile_idx][:, :, :]

            def tT_producer(nc_, md):
                return tT_c[:, md.m_tile_idx : md.m_tile_idx + 1, :]

            composable_matmul_tile_kernel(
                tc=tc,
                kxm_shape=k_shape,
                kxn_shape=n_shape,
                output_type=None,
                kxm_producer=xe_producer,
                kxn_producer=at8_producer_kxn,
                mxn_subtile_reducer=scalar_copyback(),
                mxn_consumer=noop,
                mxn_subtile_producer=tT_producer,
                psum_n_bufs=2,
                MAX_K_TILE_SIZE=KQ,
            )
            m1ctx.close()
            # b1 prefetch after M1 so it doesn't starve M1's tiles in the
            # DMA queues (scheduler priority = trace order)
            if not b1_zero:
                nc.sync.dma_start(out=b1c_sb[:, :], in_=b1[:, :])

            # M3: h1T = relu((We_ext.T W1.T).T @ t.T + b1)   [HID, S]
            # feature-major so h1T is directly the kxm cache for M5;
            # kxm streamed from the two wcb half tiles
            m3_k_shape = ShapeInfo(pdims=((128, KEP // 128),), fdims=(HID,))
            m3_n_shape = ShapeInfo(pdims=((128, KEP // 128),), fdims=(S,))
            m3_ksub = 2 if M3_FP8 else 1

            def wcb_producer(nc_, md):
                # M_TILE == WCW == 512, so chunk == m_tile_idx
                return wcb_c[md.m_tile_idx][
                    :,
                    md.k_tile_idx * md.k_subtiles : (md.k_tile_idx + 1)
                    * md.k_subtiles,
                    :,
                ]

            def tT_producer_kxn(nc_, md):
                return tT_c[
                    :,
                    md.k_tile_idx * md.k_subtiles : (md.k_tile_idx + 1)
                    * md.k_subtiles,
                    :,
                ]

            def h1T_producer(nc_, md):
                return h1T_c[
                    :, MSUB * md.m_tile_idx : MSUB * (md.m_tile_idx + 1), :
                ]

            composable_matmul_tile_kernel(
                tc=tc,
                kxm_shape=m3_k_shape,
                kxn_shape=m3_n_shape,
                output_type=None,
                kxm_producer=wcb_producer,
                kxn_producer=tT_producer_kxn,
                mxn_subtile_reducer=_m3_reducer(nc, None if b1_zero else b1c_sb),
                mxn_consumer=noop,
                mxn_subtile_producer=h1T_producer,
                psum_n_bufs=2,
                MAX_K_TILE_SIZE=128 * m3_ksub,
            )

            # M5 quarters: p[:, q] = h1T.T @ w2[:, q]        [S, HQ] fp8
            # entirely local (W2 replicated); each quarter's AllGather fires
            # as soon as the quarter is in DRAM, so all 4 AGs hide under the
            # remaining M5 quarters and the early M4 quarters.
            m5_kxn_pool = octx.enter_context(
                tc.tile_pool(name="m5_kxn_pool", bufs=7)
            )
            m5_kxm_shape = ShapeInfo(pdims=((128, HID // 128),), fdims=(S,))

            def h1T_producer_kxm(nc_, md):
                return h1T_c[
                    :,
                    md.k_tile_idx * md.k_subtiles : (md.k_tile_idx + 1)
                    * md.k_subtiles,
                    md.m_tile_idx * md.m_tile : (md.m_tile_idx + 1) * md.m_tile,
                ]

            for i in range(NAG):
                kxn_producer, kxn_shape = dma_from_dram_kxn(
                    m5_kxn_pool, w2[:, i * HQ : (i + 1) * HQ]
                )
                composable_matmul_tile_kernel(
                    tc=tc,
                    kxm_shape=m5_kxm_shape,
                    kxn_shape=kxn_shape,
                    output_type=ADT,
                    kxm_producer=h1T_producer_kxm,
                    kxn_producer=kxn_producer,
                    mxn_subtile_reducer=scalar_copyback(),
                    mxn_consumer=dma_to_dram_mxn(p_c[i][:, :]),
                    psum_n_bufs=2,
                    MAX_K_TILE_SIZE=2048,
                )
                nc.gpsimd.collective_compute(
                    "AllGather",
                    mybir.AluOpType.bypass,
                    ins=[p_c[i][:, :].opt()],
                    outs=[p_f[i][:, :].opt()],
                    replica_groups=[list(range(NCORES))],
                )
            # b2 prefetch (needed by M4's reducer) after the AGs trigger so
            # the p bounce writes aren't queued behind it
            if not b2_zero:
                nc.sync.dma_start(out=b2_sb[:, :], in_=b2[:, :])

            # M4 quarters: aggr2[:, q] = at8.T @ p_full[:, q]  [S, HQ]
            # relu+b2 fused into the eviction; row-sums stream into hm_parts.
            # kxm reuses M1's resident at8 quarter tiles (no re-DMA).
            # w2 was pre-scaled by WSCALE and b2 holds WSCALE*b2, so the
            # accumulated sums are WSCALE*h2; the hm normalization divides
            # it back out.
            m4_kxn_pool = octx.enter_context(
                tc.tile_pool(name="m4_kxn_pool", bufs=10)
            )
            # head psum pool coexists with M4's (psum_n_bufs=1 there):
            # 4 + 3 banks <= 8
            hpsum = octx.enter_context(
                tc.tile_pool(name="hpsum", bufs=2, space="PSUM")
            )
            m4_kxm_shape = ShapeInfo(pdims=((128, N // 128),), fdims=(S,))

            def at8_producer_kxm(nc_, md):
                return at8_q[md.k_tile_idx][
                    :, :, md.m_tile_idx * md.m_tile : (md.m_tile_idx + 1) * md.m_tile
                ]

            m4_kxn_shape = ShapeInfo(pdims=((128, N // 128),), fdims=(HQ,))

            def m4_kxn_producer_for(pf_ap):
                # custom kxn stream on the ACT ring: p_f reads must not sit
                # behind p_c eviction writes / w2 tiles on the SP ring
                pf_r = pf_ap.rearrange("(po pi) n -> pi po n", pi=128)

                def _producer(nc_, md):
                    t = m4_kxn_pool.tile(
                        [128, md.k_subtiles, md.n_tile], ADT, tag="m4kxn"
                    )
                    nc_.scalar.dma_start(
                        out=t[:],
                        in_=pf_r[
                            :,
                            md.k_tile_idx * md.k_subtiles : (md.k_tile_idx + 1)
                            * md.k_subtiles,
                            md.n_tile_idx * md.n_tile : (md.n_tile_idx + 1)
                            * md.n_tile,
                        ],
                    )
                    return t

                return _producer

            NB = CHID // 512  # 4 zp column blocks
            for i in range(NAG):
                kxn_producer = m4_kxn_producer_for(p_f[i][:, :])
                kxn_shape = m4_kxn_shape
                composable_matmul_tile_kernel(
                    tc=tc,
                    kxm_shape=m4_kxm_shape,
                    kxn_shape=kxn_shape,
                    output_type=FP8,
                    kxm_producer=at8_producer_kxm,
                    kxn_producer=kxn_producer,
                    mxn_subtile_reducer=_m4_reducer(
                        nc, None if b2_zero else b2_sb, hm_parts, i * NTQ
                    ),
                    mxn_consumer=noop,
                    psum_n_bufs=1,
                    MAX_K_TILE_SIZE=KQ,
                )
                if i == 0:
                    nc.scalar.dma_start(
                        out=wc1_t[:, :, :],
                        in_=wc1[:, :].rearrange("(po pi) n -> pi po n", pi=128),
                    )
                    nc.scalar.dma_start(out=bc1_t[:, :], in_=bc1[:, :])
                    nc.scalar.dma_start(out=wc2_t[:, :], in_=wc2[:, :])
                # z chunk after quarters 1 and 3: this chunk's hm
                # contribution -> zp -> AllGather (half the AllReduce
                # latency); chunk 0's AG rides under M4 quarters 2-3 and
                # only chunk 1's AG sits on the tail
                if i % (NAG // NZ) != (NAG // NZ) - 1:
                    continue
                iz = i // (NAG // NZ)
                nc.vector.tensor_reduce(
                    out=hm_i[iz][:, :],
                    in_=hm_parts[:, :, iz * NTZ : (iz + 1) * NTZ],
                    axis=mybir.AxisListType.X, op=mybir.AluOpType.add,
                )
                nc.vector.tensor_scalar_mul(
                    hm_i[iz][:, :], hm_i[iz][:, :],
                    1.0 / (HID * (WSCALE if USE_FP8 else 1.0)),
                )
                nc.vector.tensor_copy(out=hm_ib[iz][:, :], in_=hm_i[iz][:, :])
                for j in range(NB):
                    psj = hpsum.tile([128, 512], F32, name="zpps")
                    for ko in range(MSUB):
                        nc.tensor.matmul(
                            psj[0:1, :],
                            hm_ib[iz][:, ko : ko + 1],
                            wc1_t[:, ko, 512 * j : 512 * (j + 1)],
                            start=(ko == 0),
                            stop=(ko == MSUB - 1),
                        )
                    nc.vector.tensor_copy(
                        out=zp_t[iz][:, 512 * j : 512 * (j + 1)], in_=psj[0:1, :]
                    )
                nc.sync.dma_start(out=zgb[iz][:, :], in_=zp_t[iz][:, :])
                nc.gpsimd.collective_compute(
                    "AllGather",
                    mybir.AluOpType.bypass,
                    ins=[zgb[iz][:, :].opt()],
                    outs=[zgf[iz][:, :].opt()],
                    replica_groups=[list(range(NCORES))],
                )
                if iz == 0:
                    # pre-stage: zgf0 lands mid-M4q2, so sum the 8 core
                    # contributions and fold bc1 now, leaving only chunk
                    # 1's sum on the post-AG tail
                    for g in range(NCORES):
                        nc.sync.dma_start(
                            out=zg_t[0][:, g : g + 1, :],
                            in_=zgf[0][g : g + 1, :].rearrange(
                                "o (p i) -> p (o i)", p=128
                            ),
                        )
                    nc.vector.tensor_add(
                        out=z2_t[:, :], in0=zg_t[0][:, 0, :], in1=zg_t[0][:, 1, :]
                    )
                    for g in range(2, NCORES):
                        nc.vector.tensor_add(
                            out=z2_t[:, :], in0=z2_t[:, :], in1=zg_t[0][:, g, :]
                        )
                    nc.vector.tensor_add(
                        out=z2_t[:, :], in0=z2_t[:, :], in1=bc1_t[:, :]
                    )
            # epilogue on z viewed as [128, 16] so the DVE ops use all lanes
            psr = hpsum.tile([128, 512], F32, name="zpps")
            for g in range(NCORES):
                nc.sync.dma_start(
                    out=zg_t[1][:, g : g + 1, :],
                    in_=zgf[1][g : g + 1, :].rearrange("o (p i) -> p (o i)", p=128),
                )
            for g in range(NCORES):
                nc.vector.tensor_add(
                    out=z2_t[:, :], in0=z2_t[:, :], in1=zg_t[1][:, g, :]
                )
            nc.vector.tensor_scalar_max(z2_t[:, :], z2_t[:, :], 0.0)
            nc.vector.tensor_mul(out=z2_t[:, :], in0=z2_t[:, :], in1=wc2_t[:, :])
            nc.vector.tensor_reduce(
                out=zcol_t[:, :], in_=z2_t[:, :],
                axis=mybir.AxisListType.X, op=mybir.AluOpType.add,
            )
            # cross-partition sum via a 128x1 ones matmul
            nc.tensor.matmul(
                psr[0:1, 0:1], ones_t[:, 0:1], zcol_t[:, 0:1], start=True, stop=True
            )
            nc.vector.tensor_copy(out=r_t[:, :], in_=psr[0:1, 0:1])
            nc.sync.dma_start(out=res[:, :], in_=r_t[:, :])

    nc.compile()
    nc.m = get_hw_module(nc.m)
    return nc


def get_compiled(b1_zero=True, b2_zero=True):
    key = (b1_zero, b2_zero)
    if key not in _COMPILED:
        _COMPILED[key] = _build_graph(*key)
    return _COMPILED[key]


def _bf16(a):
    return np.ascontiguousarray(np.asarray(a, dtype=np.float32)).astype(ml_dtypes.bfloat16)


def _f32(a):
    return np.ascontiguousarray(np.asarray(a, dtype=np.float32))


_NP_FP8 = mybir.dt.np(FP8)


def _adt(a):
    """Convert to the aggregation dtype (fp8 or bf16)."""
    a = np.ascontiguousarray(np.asarray(a, dtype=np.float32))
    return a.astype(_NP_FP8 if USE_FP8 else ml_dtypes.bfloat16)


def make_in_maps(x, edge_index, W_embed, b_embed, W1, b1, W2, b2, Wc1, bc1, Wc2, bc2):
    x = _f32(x)
    ei = np.asarray(edge_index).astype(np.int64)
    # adjacency counts, transposed: AT[src, dst] = #edges src->dst
    counts = np.bincount(ei[1] * N + ei[0], minlength=N * N).astype(np.float32)
    AT = counts.reshape(N, N)

    # padded to KEP so M1 computes the tT DoubleRow-pad rows as real zeros
    x_ext = np.zeros((N, KEP), np.float32)
    x_ext[:, :IN_DIM] = x
    x_ext[:, IN_DIM] = 1.0

    we_ext = np.zeros((KEP, HID), np.float32)
    we_ext[:IN_DIM] = _f32(W_embed).T
    we_ext[IN_DIM] = _f32(b_embed)
    # layer-1 transform is low-rank: fold We_ext.T @ W1.T on the host
    wcb_full = we_ext @ _f32(W1).T
    if M3_FP8:
        # scale into e4m3's normal range; h1T then carries WCB_SCALE and
        # the p eviction divides it back out
        wcb_np = _adt(wcb_full * WCB_SCALE)
    else:
        wcb_np = _bf16(wcb_full)

    xe_np = _adt(x_ext)
    at8_np = _adt(AT)
    wmul = WSCALE if USE_FP8 else 1.0
    w2_np = _adt(_f32(W2).T * wmul) if USE_FP8 else _bf16(_f32(W2).T)
    # b1 per-partition layout for the feature-major h1T eviction (h1T
    # carries the WCB_SCALE factor, so b1 must too)
    b1s = _f32(b1) * (WCB_SCALE if M3_FP8 else 1.0)
    b1c_np = _f32(np.ascontiguousarray(b1s.reshape(HID // 128, 128).T))
    b2s = _f32(b2) * (WSCALE if USE_FP8 else 1.0)
    b2_np = _f32(np.broadcast_to(b2s, (128, HID)))
    wc1T = _bf16(_f32(Wc1).T)  # [HID(nodes), CHID] bf16
    wc2_row = _f32(Wc2).reshape(128, CHID // 128)
    bc1_full = _f32(bc1).reshape(128, CHID // 128)

    in_maps = []
    for c in range(NCORES):
        rows = slice(S * c, S * (c + 1))
        in_maps.append(
            {
                "xe": xe_np,
                "wcb": wcb_np,
                "at8": np.ascontiguousarray(at8_np[:, rows]),
                "w2": w2_np,
                "b1": b1c_np,
                "b2": b2_np,
                "wc1": np.ascontiguousarray(wc1T[rows, :]),
                "bc1": bc1_full,
                "wc2": wc2_row,
            }
        )
    return in_maps


def kernel(**inputs):
    b1_zero = not np.any(np.asarray(inputs["b1"], dtype=np.float32))
    b2_zero = not np.any(np.asarray(inputs["b2"], dtype=np.float32))
    nc = get_compiled(b1_zero, b2_zero)
    in_maps = make_in_maps(**inputs)
    bres = run_bass_kernel_spmd(nc, in_maps, core_ids=list(range(NCORES)))
    val = np.float32(bres.results[0]["res"][0, 0])
    bc2 = np.asarray(inputs["bc2"], dtype=np.float32).reshape(-1)
    out = np.asarray(val + bc2[0], dtype=np.float32).reshape(())
    return out


# revision 21
# speedup vs baseline: 1.0587x; 1.0587x over previous
"""Trainium2 Bass kernel for a 2-layer message-passing GNN (BaselineGNN).

Reference computation (N=4096 nodes, IN=512, HID=4096, E=65536 edges):
    h   = x @ We.T + be                                   [N, HID]
    for W, b in ((W1, b1), (W2, b2)):
        aggr = segment_sum(h[col], row)                   [N, HID]
        h    = relu(aggr @ W.T + b)
    hm  = mean(h, axis=1)                                 [N]
    z   = relu(hm @ Wc1.T + bc1)                          [HID//2]
    out = (z @ Wc2.T + bc2).squeeze(-1)                   scalar

Strategy (8 NeuronCores, node-parallel, deep collective pipelining):
  * segment_sum == A @ h with A the [N, N] adjacency-count matrix (0.4%
    dense).  A's entries are small integer counts -> exactly representable
    in fp8-e4m3, so aggregation runs as a dense TensorEngine matmul.
  * Nodes are sharded: core c owns rows 512c..512c+512.
  * Layer 1 is low-rank through the embed bottleneck:
        h1_c = relu((A_c @ x_ext) @ (We_ext.T @ W1.T) + b1)
    with the weight product folded on the host (x_ext carries a ones
    column, We_ext.T a b_embed row).
  * Layer 2 is REORDERED to hide the collective:
        h2 = relu(A @ (h1 @ W2.T) + b2)
    Each core computes p_c = h1_c @ W2.T locally (W2 replicated) BEFORE
    any cross-core exchange.  p is exchanged in NAG=4 column quarters:
    each quarter's AllGather fires as soon as M5 finishes that quarter,
    so all four AGs ride under the remaining M5 quarters and early M4
    quarters -- the PE never waits for the fabric.
  * Pipeline per core: M1 tT=(A_c@x_ext).T [fp8 DR, ONE matmul k-batched
    over 4 quarter-tiles so the first matmul depends on only ~1.1MB of
    DMA] -> M3 h1T [bf16, kxm streamed from two wcb half-tiles so M3
    starts when half the weight product has landed] -> M5 quarters
    p_c = h1T.T @ (W2.T*16) [fp8 DR] -> AllGather quarter (fp8) ->
    M4 quarters aggr2 = at8.T @ p_full [fp8 DR] with relu+row-sum fused
    into the PSUM eviction (ScalarE accum_out) -> hm directly.
  * M4's kxm side reuses M1's at8 quarter tiles (still resident in
    SBUF), so the adjacency matrix is DMAed exactly once.
  * The z head runs in 2 chunks (after M4 quarters 1 and 3); each chunk
    zp = Wc1_c @ hm_chunk is exchanged with an AllGather (half the
    latency of AllReduce = RS+AG) and summed locally on the DVE, so
    only the last small AG sits on the tail.
  * fp8 operands that are not exact (x, h1, p, W2) rely on fp32 PSUM
    accumulation and the mean-pool/classifier averaging to wash the 6%
    element-level rounding down to ~1e-3 relative error on the scalar
    output; W2 is pre-scaled by 16 to sit in e4m3's normal range (the
    hm normalization divides it back out).  The classifier runs bf16/fp32.
  * A dummy AllReduce issued first absorbs the multi-core launch skew on
    the collective engine while the PE computes M1/M3/M5.

Everything is expressed "transposed-free": every matmul is
mxn = kxm.T @ kxn with operands stored so no transposes are ever needed.
"""

import contextlib

import numpy as np
import ml_dtypes

import concourse.bass as bass
import concourse.mybir as mybir
import concourse.tile as tile
from concourse import bacc
from concourse.bass_interp import get_hw_module
from concourse.bass_utils import run_bass_kernel_spmd
from concourse.kernels.tile_matmul import (
    ShapeInfo,
    composable_matmul_tile_kernel,
    dma_from_dram_kxn,
    dma_to_dram_mxn,
    k_pool_min_bufs,
    scalar_copyback,
    scalar_scale,
)

N = 4096          # nodes
IN_DIM = 512
HID = 4096
NCORES = 8
S = N // NCORES           # nodes per core (512)
KE = 640                  # extended embed contraction (512 + 1 ones col, padded to 5*128)
CHID = HID // 2           # classifier hidden (2048)

BF16 = mybir.dt.bfloat16
F32 = mybir.dt.float32
FP8 = mybir.dt.float8e4

USE_FP8 = True
# p = h1 @ (W2.T * WSCALE) is materialized in fp8-e4m3 (range +-448):
# p_true has absmax ~12, so WSCALE=16 centers it (~200 max) with margin.
WSCALE = 16.0
M3_FP8 = False
WCB_SCALE = 16.0
KEP = 768 if M3_FP8 else KE   # KE padded to a DoubleRow-even k (6*128)

NAG = 4                   # p AllGather column-chunk count
HQ = HID // NAG           # 1024
NZ = 2                    # z-exchange chunk count
NKQ = 8                   # M1/M4 k-chunk tiles (N // 512)
KQ = N // NKQ             # 512 rows per k-chunk
NWC = 8                   # wcb column chunks

_COMPILED = {}


def _m4_reducer(nc_b, bias_sb, accum, n_off):
    """PSUM->SBUF evict for the final aggregation:
    sbuf = relu(psum + b2[n_slice]); row-sums stream into accum.

    bias_sb: [128, HID] SBUF tile (b2 broadcast across partitions), or None
    when b2 is all-zero.  accum: [128, MSUB, NT] fp32; the relu
    row-sum-accumulates into accum[:, m_subtile, n_off + n_tile].
    """

    def _reducer(nc, psum, sbuf, md):
        src_ap = psum[:, : md.n_slice_size]
        ni = n_off + md.n_tile_idx * md.n_subtiles + md.n_subtile_idx
        if bias_sb is not None:
            start = (
                n_off * md.n_tile
                + md.n_tile_idx * md.n_tile
                + md.n_subtile_idx * md.n_subtile
            )
            nc.vector.tensor_add(
                out=sbuf[:, :, : md.n_slice_size],
                in0=src_ap,
                in1=bias_sb[:, start : start + md.n_slice_size],
            )
            src_ap = sbuf
        nc.scalar.activation(
            out=sbuf,
            in_=src_ap,
            func=mybir.ActivationFunctionType.Relu,
            accum_out=accum[:, md.m_subtile_idx, ni : ni + 1],
        )

    return _reducer


def _m3_reducer(nc_b, b1c_sb):
    """PSUM->SBUF evict for h1T: sbuf = relu(psum + b1[m_slice]).

    h1T is feature-major, so the layer-1 bias is per-partition:
    b1c_sb is [128, HID//128] with b1c[p, m] = b1[m*128 + p], or None
    when b1 is all-zero.
    """

    def _reducer(nc, psum, sbuf, md):
        if b1c_sb is None:
            nc.scalar.activation(
                out=sbuf, in_=psum, func=mybir.ActivationFunctionType.Relu
            )
        else:
            mi = md.m_tile_idx * md.m_subtiles + md.m_subtile_idx
            nc.vector.tensor_add(
                out=sbuf[:, :, : md.n_slice_size],
                in0=psum[:, : md.n_slice_size],
                in1=b1c_sb[:, mi : mi + 1].to_broadcast(
                    [128, 1, md.n_slice_size]
                ),
            )
            nc.vector.tensor_scalar_max(sbuf, sbuf, 0.0)

    return _reducer


def _build_graph(b1_zero=True, b2_zero=True):
    nc = bacc.Bacc(
        "TRN2",
        target_bir_lowering=False,
        debug=False,
        enable_asserts=False,
        num_devices=NCORES,
    )

    ADT = FP8 if USE_FP8 else BF16

    # ---- kernel I/O (per core) ----
    WBT = ADT if M3_FP8 else BF16
    xe = nc.dram_tensor("xe", [N, KEP], ADT, kind="ExternalInput")        # x_ext (replicated, padded)
    at8 = nc.dram_tensor("at8", [N, S], ADT, kind="ExternalInput")        # A.T[:, rows_c] (sharded)
    wcb = nc.dram_tensor("wcb", [KEP, HID], WBT, kind="ExternalInput")    # We_ext.T @ W1.T (replicated)
    w2 = nc.dram_tensor("w2", [HID, HID], ADT, kind="ExternalInput")      # W2.T * WSCALE (replicated)
    b1 = nc.dram_tensor("b1", [128, HID // 128], F32, kind="ExternalInput")  # b1 partition-major
    b2 = nc.dram_tensor("b2", [128, HID], F32, kind="ExternalInput")      # b2*WSCALE bcast (replicated)
    wc1 = nc.dram_tensor("wc1", [S, CHID], BF16, kind="ExternalInput")    # Wc1.T row-chunk (sharded)
    bc1 = nc.dram_tensor("bc1", [128, CHID // 128], F32, kind="ExternalInput")  # bc1 [128,16]
    wc2 = nc.dram_tensor("wc2", [128, CHID // 128], F32, kind="ExternalInput")  # Wc2 [128,16]
    res = nc.dram_tensor("res", [1, 1], F32, kind="ExternalOutput")       # final scalar (pre-bc2)

    # ---- internal DRAM ----
    # p = h1 @ (W2.T*16) is exchanged in column quarters so each AllGather
    # overlaps with the production of the next quarter (M5) / consumption
    # of previous ones (M4).
    p_c = [nc.dram_tensor(f"pc{i}", [S, HQ], ADT) for i in range(NAG)]
    p_f = [
        nc.dram_tensor(f"pf{i}", [N, HQ], ADT, addr_space="Shared")
        for i in range(NAG)
    ]
    zgb = [nc.dram_tensor(f"zgb{i}", [1, CHID], F32) for i in range(NZ)]
    zgf = [
        nc.dram_tensor(f"zgf{i}", [NCORES, CHID], F32, addr_space="Shared")
        for i in range(NZ)
    ]
    da = nc.dram_tensor("da", [1, 8], F32)              # launch-skew sync dummy
    df = nc.dram_tensor("df", [1, 8], F32, addr_space="Shared")

    MSUB = S // 128   # 4 m-subtiles in a 512-row tile
    NT = HID // 512   # 8 hm column groups of 512
    NTQ = HQ // 512   # n-tiles (hm groups) per AG quarter (2)
    NTZ = NT // NZ    # hm groups per z chunk (4)
    KSQ = KQ // 128   # k-subtiles per k-quarter tile (8)

    with tile.TileContext(nc) as tc:
        with contextlib.ExitStack() as octx:
            const = octx.enter_context(tc.tile_pool(name="const", bufs=1))
            b2_sb = (
                None if b2_zero else const.tile([128, HID], F32, name="b2_sb")
            )
            b1c_sb = const.tile([128, HID // 128], F32, name="b1c_sb")
            hm_parts = const.tile([128, MSUB, NT], F32, name="hm_parts")
            nc.any.memset(hm_parts[:], 0.0)

            head = octx.enter_context(tc.tile_pool(name="head", bufs=1))
            CI = CHID // 128  # 16
            wc1_t = head.tile([128, MSUB, CHID], BF16, name="wc1_t")
            bc1_t = head.tile([128, CI], F32, name="bc1_t")
            wc2_t = head.tile([128, CI], F32, name="wc2_t")
            zp_t = [head.tile([1, CHID], F32, name=f"zp_t{i}") for i in range(NZ)]
            hm_i = [head.tile([128, MSUB], F32, name=f"hm_i{i}") for i in range(NZ)]
            hm_ib = [head.tile([128, MSUB], BF16, name=f"hm_ib{i}") for i in range(NZ)]
            zg_t = [
                head.tile([128, NCORES, CI], F32, name=f"zg_t{i}")
                for i in range(NZ)
            ]
            z2_t = head.tile([128, CI], F32, name="z2_t")
            zcol_t = head.tile([128, 1], F32, name="zcol_t")
            ones_t = head.tile([128, 1], F32, name="ones_t")
            r_t = head.tile([1, 1], F32, name="r_t")
            nc.any.memset(ones_t[:], 1.0)

            # persistent SBUF caches for chained-matmul intermediates.
            # at8 quarters stay resident: M1's kxn side AND M4's kxm side.
            cache = octx.enter_context(tc.tile_pool(name="cache", bufs=1))
            at8_q = [
                cache.tile([128, KSQ, S], ADT, name=f"at8_q{i}")
                for i in range(NKQ)
            ]
            tT_c = cache.tile([128, KEP // 128, S], WBT, name="tT_c")
            h1T_c = cache.tile([128, HID // 128, S], ADT, name="h1T_c")
            # wcb in column chunks (one per M3 m-tile) so M3's first
            # m-tiles depend on only ~0.6MB of the 5MB weight product
            WCW = HID // NWC  # 512
            wcb_c = [
                cache.tile([128, KEP // 128, WCW], WBT, name=f"wcb_c{i}")
                for i in range(NWC)
            ]
            # xe chunks live only until M1 completes
            m1ctx = contextlib.ExitStack()
            m1pool = m1ctx.enter_context(tc.tile_pool(name="m1pool", bufs=1))
            xe_q = [
                m1pool.tile([128, KSQ, KEP], ADT, name=f"xe_q{i}")
                for i in range(NKQ)
            ]
            at8_r = at8[:, :].rearrange("(po pi) n -> pi po n", pi=128)
            xe_r = xe[:, :].rearrange("(po pi) n -> pi po n", pi=128)
            wcb_r = wcb[:, :].rearrange("(po pi) n -> pi po n", pi=128)
            # interleave chunk loads, alternating rings per k-chunk so both
            # rings deliver M1's operands in consumption order; the first
            # matmul depends on only ~0.6MB
            for q in range(NKQ):
                po = q * KSQ
                ea = nc.sync if q % 2 == 0 else nc.scalar
                eb = nc.scalar if q % 2 == 0 else nc.sync
                ea.dma_start(
                    out=at8_q[q][:, :, :], in_=at8_r[:, po : po + KSQ, :]
                )
                eb.dma_start(
                    out=xe_q[q][:, :, :], in_=xe_r[:, po : po + KSQ, :]
                )
            # wcb chunks next, alternating rings, in M3 m-tile order
            for h in range(NWC):
                eng = nc.sync if h % 2 == 0 else nc.scalar
                eng.dma_start(
                    out=wcb_c[h][:, :, :],
                    in_=wcb_r[:, :, h * WCW : (h + 1) * WCW],
                )
            noop = lambda nc_, sbuf, md: None

            # dummy AllReduce fired first: absorbs the multi-core launch skew
            # on the collective engine while the PE is busy with M1/M3/M5, so
            # the p AllGathers later only see compute drift
            sync_t = head.tile([1, 8], F32, name="sync_t")
            nc.any.memset(sync_t[:], 0.0)
            nc.sync.dma_start(out=da[:, :], in_=sync_t[:, :])
            nc.gpsimd.collective_compute(
                "AllReduce",
                mybir.AluOpType.add,
                ins=[da[:, :].opt()],
                outs=[df[:, :].opt()],
                replica_groups=[list(range(NCORES))],
            )

            # M1: tT = (A_c @ x_ext).T = xe.T @ A_c.T        [KEP, S]
            # one matmul, k-batched over the 4 quarter tiles (PSUM
            # accumulates across k-tiles), so no combine pass is needed and
            # the first matmul depends on only the first quarters' DMAs
            k_shape = ShapeInfo(pdims=((128, N // 128),), fdims=(KEP,))
            n_shape = ShapeInfo(pdims=((128, N // 128),), fdims=(S,))

            def xe_producer(nc_, md):
                return xe_q[md.k_tile_idx][
                    :, :, md.m_tile_idx * md.m_tile : (md.m_tile_idx + 1) * md.m_tile
                ]

            def at8_producer_kxn(nc_, md):
                return at8_q[md.k_tile_idx][:, :, :]

            def tT_producer(nc_, md):
                return tT_c[
                    :,
                    md.m_tile_idx * md.m_subtiles : (md.m_tile_idx + 1)
                    * md.m_subtiles,
                    :,
                ]

            composable_matmul_tile_kernel(
                tc=tc,
                kxm_shape=k_shape,
                kxn_shape=n_shape,
                output_type=None,
                kxm_producer=xe_producer,
                kxn_producer=at8_producer_kxn,
                mxn_subtile_reducer=scalar_copyback(),
                mxn_consumer=noop,
                mxn_subtile_producer=tT_producer,
                psum_n_bufs=2,
                MAX_K_TILE_SIZE=KQ,
            )
            m1ctx.close()
            # b1 prefetch after M1 so it doesn't starve M1's tiles in the
            # DMA queues (scheduler priority = trace order)
            if not b1_zero:
                nc.sync.dma_start(out=b1c_sb[:, :], in_=b1[:, :])

            # M3: h1T = relu((We_ext.T W1.T).T @ t.T + b1)   [HID, S]
            # feature-major so h1T is directly the kxm cache for M5;
            # kxm streamed from the two wcb half tiles
            m3_k_shape = ShapeInfo(pdims=((128, KEP // 128),), fdims=(HID,))
            m3_n_shape = ShapeInfo(pdims=((128, KEP // 128),), fdims=(S,))
            m3_ksub = 2 if M3_FP8 else 1

            def wcb_producer(nc_, md):
                # M_TILE == WCW == 512, so chunk == m_tile_idx
                return wcb_c[md.m_tile_idx][
                    :,
                    md.k_tile_idx * md.k_subtiles : (md.k_tile_idx + 1)
                    * md.k_subtiles,
                    :,
                ]

            def tT_producer_kxn(nc_, md):
                return tT_c[
                    :,
                    md.k_tile_idx * md.k_subtiles : (md.k_tile_idx + 1)
                    * md.k_subtiles,
                    :,
                ]

            def h1T_producer(nc_, md):
                return h1T_c[
                    :, MSUB * md.m_tile_idx : MSUB * (md.m_tile_idx + 1), :
                ]

            composable_matmul_tile_kernel(
                tc=tc,
                kxm_shape=m3_k_shape,
                kxn_shape=m3_n_shape,
                output_type=None,
                kxm_producer=wcb_producer,
                kxn_producer=tT_producer_kxn,
                mxn_subtile_reducer=_m3_reducer(nc, None if b1_zero else b1c_sb),
                mxn_consumer=noop,
                mxn_subtile_producer=h1T_producer,
                psum_n_bufs=2,
                MAX_K_TILE_SIZE=128 * m3_ksub,
            )

            # M5 quarters: p[:, q] = h1T.T @ w2[:, q]        [S, HQ] fp8
            # entirely local (W2 replicated); each quarter's AllGather fires
            # as soon as the quarter is in DRAM, so all 4 AGs hide under the
            # remaining M5 quarters and the early M4 quarters.
            # With M3_FP8 both h1T and w2 carry a x16 scale; the eviction
            # divides one factor out so p stays WSCALE-scaled.
            m5_evict = (
                scalar_scale(1.0 / WCB_SCALE) if M3_FP8 else scalar_copyback()
            )
            m5_kxn_pool = octx.enter_context(
                tc.tile_pool(name="m5_kxn_pool", bufs=7)
            )
            # M4's stream pool + head psum are created BEFORE the M5 loop so
            # their SBUF/PSUM ranges are disjoint from M5's eviction temps --
            # otherwise M4's first kxn DMAs inherit a WAR hazard on M5q3's
            # temps and the whole M4 pipeline start waits for the last p_c
            # write to drain.
            m4_kxn_pool = octx.enter_context(
                tc.tile_pool(name="m4_kxn_pool", bufs=16)
            )
            # M4 kxn prefetch: all p_f tile loads are emitted on the (idle)
            # GPSIMD ring, interleaved with the AG triggers below, so each
            # quarter's tiles start loading the instant its AllGather lands
            # -- by the time the PE finishes M5 the M4 stream is deep in
            # SBUF and the M5->M4 transition costs ~nothing.  Pool rotation
            # (bufs=20 of 32 total tiles) provides the flow control.
            m4_tiles = {}

            def emit_m4_prefetch(q):
                pf_r = p_f[q].rearrange("(po pi) n -> pi po n", pi=128)
                for nt in range(HQ // 512):
                    for kt in range(NKQ):
                        t = m4_kxn_pool.tile(
                            [128, KSQ, 512], ADT, tag="m4kxn"
                        )
                        nc.gpsimd.dma_start(
                            out=t[:],
                            in_=pf_r[
                                :,
                                kt * KSQ : (kt + 1) * KSQ,
                                nt * 512 : (nt + 1) * 512,
                            ],
                        )
                        m4_tiles[(q, nt, kt)] = t
            m5_kxm_shape = ShapeInfo(pdims=((128, HID // 128),), fdims=(S,))

            def h1T_producer_kxm(nc_, md):
                return h1T_c[
                    :,
                    md.k_tile_idx * md.k_subtiles : (md.k_tile_idx + 1)
                    * md.k_subtiles,
                    md.m_tile_idx * md.m_tile : (md.m_tile_idx + 1) * md.m_tile,
                ]

            for i in range(NAG):
                kxn_producer, kxn_shape = dma_from_dram_kxn(
                    m5_kxn_pool, w2[:, i * HQ : (i + 1) * HQ]
                )
                composable_matmul_tile_kernel(
                    tc=tc,
                    kxm_shape=m5_kxm_shape,
                    kxn_shape=kxn_shape,
                    output_type=ADT,
                    kxm_producer=h1T_producer_kxm,
                    kxn_producer=kxn_producer,
                    mxn_subtile_reducer=m5_evict,
                    mxn_consumer=dma_to_dram_mxn(p_c[i][:, :]),
                    psum_n_bufs=2,
                    MAX_K_TILE_SIZE=2048,
                )
                nc.gpsimd.collective_compute(
                    "AllGather",
                    mybir.AluOpType.bypass,
                    ins=[p_c[i][:, :].opt()],
                    outs=[p_f[i][:, :].opt()],
                    replica_groups=[list(range(NCORES))],
                )
                emit_m4_prefetch(i)
            # b2 prefetch (needed by M4's reducer) after the AGs trigger so
            # the p bounce writes aren't queued behind it
            if not b2_zero:
                nc.sync.dma_start(out=b2_sb[:, :], in_=b2[:, :])
            # head psum pool coexists with M4's (psum_n_bufs=1 there):
            # 4 + 2 banks <= 8; created after the M5 loop so M5's 8-bank
            # psum pool fits
            hpsum = octx.enter_context(
                tc.tile_pool(name="hpsum", bufs=2, space="PSUM")
            )

            # M4 quarters: aggr2[:, q] = at8.T @ p_full[:, q]  [S, HQ]
            # relu+b2 fused into the eviction; row-sums stream into hm_parts.
            # kxm reuses M1's resident at8 quarter tiles (no re-DMA).
            # w2 was pre-scaled by WSCALE and b2 holds WSCALE*b2, so the
            # accumulated sums are WSCALE*h2; the hm normalization divides
            # it back out.
            m4_kxm_shape = ShapeInfo(pdims=((128, N // 128),), fdims=(S,))

            def at8_producer_kxm(nc_, md):
                return at8_q[md.k_tile_idx][
                    :, :, md.m_tile_idx * md.m_tile : (md.m_tile_idx + 1) * md.m_tile
                ]

            m4_kxn_shape = ShapeInfo(pdims=((128, N // 128),), fdims=(HQ,))

            def m4_kxn_producer_for(q):
                def _producer(nc_, md):
                    return m4_tiles[(q, md.n_tile_idx, md.k_tile_idx)][:]

                return _producer

            NB = CHID // 512  # 4 zp column blocks
            for i in range(NAG):
                kxn_producer = m4_kxn_producer_for(i)
                kxn_shape = m4_kxn_shape
                composable_matmul_tile_kernel(
                    tc=tc,
                    kxm_shape=m4_kxm_shape,
                    kxn_shape=kxn_shape,
                    output_type=FP8,
                    kxm_producer=at8_producer_kxm,
                    kxn_producer=kxn_producer,
                    mxn_subtile_reducer=_m4_reducer(
                        nc, None if b2_zero else b2_sb, hm_parts, i * NTQ
                    ),
                    mxn_consumer=noop,
                    psum_n_bufs=1,
                    MAX_K_TILE_SIZE=KQ,
                )
                if i == 0:
                    nc.scalar.dma_start(
                        out=wc1_t[:, :, :],
                        in_=wc1[:, :].rearrange("(po pi) n -> pi po n", pi=128),
                    )
                    nc.scalar.dma_start(out=bc1_t[:, :], in_=bc1[:, :])
                    nc.scalar.dma_start(out=wc2_t[:, :], in_=wc2[:, :])
                # z chunk after quarters 1 and 3: this chunk's hm
                # contribution -> zp -> AllGather (half the AllReduce
                # latency); chunk 0's AG rides under M4 quarters 2-3 and
                # only chunk 1's AG sits on the tail
                if i % (NAG // NZ) != (NAG // NZ) - 1:
                    continue
                iz = i // (NAG // NZ)
                nc.vector.tensor_reduce(
                    out=hm_i[iz][:, :],
                    in_=hm_parts[:, :, iz * NTZ : (iz + 1) * NTZ],
                    axis=mybir.AxisListType.X, op=mybir.AluOpType.add,
                )
                nc.vector.tensor_scalar_mul(
                    hm_i[iz][:, :], hm_i[iz][:, :],
                    1.0 / (HID * (WSCALE if USE_FP8 else 1.0)),
                )
                nc.vector.tensor_copy(out=hm_ib[iz][:, :], in_=hm_i[iz][:, :])
                for j in range(NB):
                    psj = hpsum.tile([128, 512], F32, name="zpps")
                    for ko in range(MSUB):
                        nc.tensor.matmul(
                            psj[0:1, :],
                            hm_ib[iz][:, ko : ko + 1],
                            wc1_t[:, ko, 512 * j : 512 * (j + 1)],
                            start=(ko == 0),
                            stop=(ko == MSUB - 1),
                        )
                    nc.vector.tensor_copy(
                        out=zp_t[iz][:, 512 * j : 512 * (j + 1)], in_=psj[0:1, :]
                    )
                nc.sync.dma_start(out=zgb[iz][:, :], in_=zp_t[iz][:, :])
                nc.gpsimd.collective_compute(
                    "AllGather",
                    mybir.AluOpType.bypass,
                    ins=[zgb[iz][:, :].opt()],
                    outs=[zgf[iz][:, :].opt()],
                    replica_groups=[list(range(NCORES))],
                )
                if iz == 0:
                    # pre-stage: zgf0 lands mid-M4q2, so sum the 8 core
                    # contributions and fold bc1 now, leaving only chunk
                    # 1's sum on the post-AG tail
                    for g in range(NCORES):
                        nc.sync.dma_start(
                            out=zg_t[0][:, g : g + 1, :],
                            in_=zgf[0][g : g + 1, :].rearrange(
                                "o (p i) -> p (o i)", p=128
                            ),
                        )
                    nc.vector.tensor_add(
                        out=z2_t[:, :], in0=zg_t[0][:, 0, :], in1=zg_t[0][:, 1, :]
                    )
                    for g in range(2, NCORES):
                        nc.vector.tensor_add(
                            out=z2_t[:, :], in0=z2_t[:, :], in1=zg_t[0][:, g, :]
                        )
                    nc.vector.tensor_add(
                        out=z2_t[:, :], in0=z2_t[:, :], in1=bc1_t[:, :]
                    )
            # epilogue on z viewed as [128, 16] so the DVE ops use all lanes
            psr = hpsum.tile([128, 512], F32, name="zpps")
            for g in range(NCORES):
                nc.sync.dma_start(
                    out=zg_t[1][:, g : g + 1, :],
                    in_=zgf[1][g : g + 1, :].rearrange("o (p i) -> p (o i)", p=128),
                )
            for g in range(NCORES):
                nc.vector.tensor_add(
                    out=z2_t[:, :], in0=z2_t[:, :], in1=zg_t[1][:, g, :]
                )
            nc.vector.tensor_scalar_max(z2_t[:, :], z2_t[:, :], 0.0)
            nc.vector.tensor_mul(out=z2_t[:, :], in0=z2_t[:, :], in1=wc2_t[:, :])
            nc.vector.tensor_reduce(
                out=zcol_t[:, :], in_=z2_t[:, :],
                axis=mybir.AxisListType.X, op=mybir.AluOpType.add,
            )
            # cross-partition sum via a 128x1 ones matmul
            nc.tensor.matmul(
                psr[0:1, 0:1], ones_t[:, 0:1], zcol_t[:, 0:1], start=True, stop=True
            )
            nc.vector.tensor_copy(out=r_t[:, :], in_=psr[0:1, 0:1])
            nc.sync.dma_start(out=res[:, :], in_=r_t[:, :])

    nc.compile()
    nc.m = get_hw_module(nc.m)
    return nc


def get_compiled(b1_zero=True, b2_zero=True):
    key = (b1_zero, b2_zero)
    if key not in _COMPILED:
        _COMPILED[key] = _build_graph(*key)
    return _COMPILED[key]


def _bf16(a):
    return np.ascontiguousarray(np.asarray(a, dtype=np.float32)).astype(ml_dtypes.bfloat16)


def _f32(a):
    return np.ascontiguousarray(np.asarray(a, dtype=np.float32))


_NP_FP8 = mybir.dt.np(FP8)


def _adt(a):
    """Convert to the aggregation dtype (fp8 or bf16)."""
    a = np.ascontiguousarray(np.asarray(a, dtype=np.float32))
    return a.astype(_NP_FP8 if USE_FP8 else ml_dtypes.bfloat16)


def make_in_maps(x, edge_index, W_embed, b_embed, W1, b1, W2, b2, Wc1, bc1, Wc2, bc2):
    x = _f32(x)
    ei = np.asarray(edge_index).astype(np.int64)
    # adjacency counts, transposed: AT[src, dst] = #edges src->dst
    counts = np.bincount(ei[1] * N + ei[0], minlength=N * N).astype(np.float32)
    AT = counts.reshape(N, N)

    # padded to KEP so M1 computes the tT DoubleRow-pad rows as real zeros
    x_ext = np.zeros((N, KEP), np.float32)
    x_ext[:, :IN_DIM] = x
    x_ext[:, IN_DIM] = 1.0

    we_ext = np.zeros((KEP, HID), np.float32)
    we_ext[:IN_DIM] = _f32(W_embed).T
    we_ext[IN_DIM] = _f32(b_embed)
    # layer-1 transform is low-rank: fold We_ext.T @ W1.T on the host
    wcb_full = we_ext @ _f32(W1).T
    if M3_FP8:
        # scale into e4m3's normal range; h1T then carries WCB_SCALE and
        # the p eviction divides it back out
        wcb_np = _adt(wcb_full * WCB_SCALE)
    else:
        wcb_np = _bf16(wcb_full)

    xe_np = _adt(x_ext)
    at8_np = _adt(AT)
    wmul = WSCALE if USE_FP8 else 1.0
    w2_np = _adt(_f32(W2).T * wmul) if USE_FP8 else _bf16(_f32(W2).T)
    # b1 per-partition layout for the feature-major h1T eviction (h1T
    # carries the WCB_SCALE factor, so b1 must too)
    b1s = _f32(b1) * (WCB_SCALE if M3_FP8 else 1.0)
    b1c_np = _f32(np.ascontiguousarray(b1s.reshape(HID // 128, 128).T))
    b2s = _f32(b2) * (WSCALE if USE_FP8 else 1.0)
    b2_np = _f32(np.broadcast_to(b2s, (128, HID)))
    wc1T = _bf16(_f32(Wc1).T)  # [HID(nodes), CHID] bf16
    wc2_row = _f32(Wc2).reshape(128, CHID // 128)
    bc1_full = _f32(bc1).reshape(128, CHID // 128)

    in_maps = []
    for c in range(NCORES):
        rows = slice(S * c, S * (c + 1))
        in_maps.append(
            {
                "xe": xe_np,
                "wcb": wcb_np,
                "at8": np.ascontiguousarray(at8_np[:, rows]),
                "w2": w2_np,
                "b1": b1c_np,
                "b2": b2_np,
                "wc1": np.ascontiguousarray(wc1T[rows, :]),
                "bc1": bc1_full,
                "wc2": wc2_row,
            }
        )
    return in_maps


def kernel(**inputs):
    b1_zero = not np.any(np.asarray(inputs["b1"], dtype=np.float32))
    b2_zero = not np.any(np.asarray(inputs["b2"], dtype=np.float32))
    nc = get_compiled(b1_zero, b2_zero)
    in_maps = make_in_maps(**inputs)
    bres = run_bass_kernel_spmd(nc, in_maps, core_ids=list(range(NCORES)))
    val = np.float32(bres.results[0]["res"][0, 0])
    bc2 = np.asarray(inputs["bc2"], dtype=np.float32).reshape(-1)
    out = np.asarray(val + bc2[0], dtype=np.float32).reshape(())
    return out
